# revision 1
# baseline (speedup 1.0000x reference)
"""Trainium2 Bass kernel for the BSplineBasis (KAN-style) layer.

Math:
  out[b,o] = sum_{i,k} C[o,i,k]*scale[o]*basis_k(clip(x[b,i])) + sum_i W[o,i]*x[b,i] + bias[o]

One fused matmul over 12 feature planes per input channel i:
  planes 0..10: fp8 spline planes g_k = -(6*basis_k)  (cubic cardinal B-spline)
  plane  11  : raw x in bf16 (residual)
Host-folded weights (SCALE=8192 lifts spline weights out of fp8 subnormals;
epilogue multiplies PSUM by 1/SCALE):
  spline rows (fp8 e4m3): W[k*I+i, o] = -(C[o,i,k]*scale[o] + bias[o]/I)/6 * SCALE
  residual rows (bf16):   W[o,i] * SCALE

Closed-form basis (uniform knots, h=0.25, s = 4*xc+4 in [0,8], d = |s-(k-1)|):
  6*basis_k = relu(2-d)^3 - 4*relu(1-d)^3
Per plane, tiles [128, 4096] processed in halves:
  ACT: dT = Abs(4*xc + (5-k))   (edge k in {0,1,9,10}: monotone -> Copy)
  ACT: b2u = Square(2-2*dT) = 4(1-d)^2 (unclamped; k=0,10 have no b-side)
  DVE: a1n = min(dT-2,0); bn = min(dT-1,0)
  a-cube: ACT3 planes: a2u = ACT Square(dT-2);   An = TT(a2u, a1n)
          ACT2 planes: a2  = TT(a1n, a1n);       An = TT(a2, a1n)
  DVE: B = TT(b2u, bn)  (= min(4(d-1)^3, 0): unclamped square x clamped lin)
  DVE: G = TT(An, B, sub) -> bf16  (= -6*basis_k; k=0,10: G = An)
  SWDGE DMA casts G bf16 -> fp8 plane tile (engine-free cast)
The ACT3/ACT2 split balances ACT vs DVE load (measured: ACT op 2.0us/half,
DVE TT 1.21, TS 0.68; STT is 1x-only so unused).

PE (batch-sharded, 512 rows/core): residual chunks first in bf16, then 44
fp8 DoubleRow chunk-pairs (contraction 256/pair): stationary = plane slice
[128,2,128b], moving = weights [128,2,512o], accumulating [128b x 512o] x 8
PSUM banks across all positions.
"""

import numpy as np
import ml_dtypes

B, I, O, K = 4096, 1024, 1024, 11
NCORES = 8
BS = B // NCORES          # 512 batch rows per core
NCH = I // 128            # 8 i-chunks per plane
NPAIRS = K * NCH // 2     # 44 fp8 chunk-pairs
FD = NCH * BS             # 4096 free dim of plane tiles: (i_chunk, b)
SCALE = 8192.0
ACT3 = {1, 3, 4, 6, 7, 9}  # planes whose a-square comes from ACT (load balance)
GPSG = set()                  # planes whose G-subtract runs on gpsimd (fp8 out)

F8 = ml_dtypes.float8_e4m3  # TRN FP8_EXP4 (max 240)

_cache = {}


def _build_bass():
    import concourse.bass as bass
    import concourse.tile as tile
    from concourse import bacc, mybir
    from contextlib import ExitStack

    F32 = mybir.dt.float32
    BF16 = mybir.dt.bfloat16
    FP8 = mybir.dt.float8e4
    AL = mybir.AluOpType
    AF = mybir.ActivationFunctionType
    DR = mybir.MatmulPerfMode.DoubleRow

    nc = bacc.Bacc("TRN2", debug=False, num_devices=NCORES)

    need = {float(v) for v in range(-5, 6)}
    for v in sorted(need):
        key = (F32, v)
        if key not in nc.const_aps.aps:
            t = nc.alloc_sbuf_tensor(f"constb-{v}", [128, 1], F32)
            nc.vector.memset(t.ap(), v)
            nc.const_aps.aps[key] = t.ap()
    # Only ACT reads the const APs (activation bias pointers); a narrow
    # DVE->Activation barrier keeps both DMA queues (sync, gpsimd) free to
    # start streaming immediately.
    nc.multi_engine_barrier([mybir.EngineType.DVE, mybir.EngineType.Activation])

    xt = nc.dram_tensor("xt", [I, BS], F32, kind="ExternalInput")
    wsp = nc.dram_tensor("wsp", [NPAIRS * 128, 2 * O], FP8, kind="ExternalInput")
    wres = nc.dram_tensor("wres", [I, O], BF16, kind="ExternalInput")
    out = nc.dram_tensor("out", [BS, O], F32, kind="ExternalOutput")

    with tile.TileContext(nc) as tc, ExitStack() as ctx:
        xpool = ctx.enter_context(tc.tile_pool(name="x", bufs=1))
        fpool = ctx.enter_context(tc.tile_pool(name="f", bufs=4))
        rpool = ctx.enter_context(tc.tile_pool(name="r", bufs=1))
        dpool = ctx.enter_context(tc.tile_pool(name="d", bufs=2))
        tpool = ctx.enter_context(tc.tile_pool(name="t", bufs=2))
        gpool = ctx.enter_context(tc.tile_pool(name="g", bufs=2))
        wspool = ctx.enter_context(tc.tile_pool(name="ws", bufs=3))
        wrpool = ctx.enter_context(tc.tile_pool(name="wr", bufs=8))
        opool = ctx.enter_context(tc.tile_pool(name="o", bufs=8))
        pspool = ctx.enter_context(tc.tile_pool(name="ps", bufs=1, space="PSUM"))

        # ---- x load: per-chunk f32 (rolling staging buffer) + casted bf16
        # residual plane (SWDGE dtype-cast during DMA, no engine time) ----
        xsb = xpool.tile([128, FD], F32, tag="xsb")
        xc = xpool.tile([128, FD], F32, tag="xc")
        fres = rpool.tile([128, NCH, BS], BF16, tag="fres")
        # x chunks stream with only the first two wres chunks interleaved;
        # wres c2..c7 follow (consumed later by the residual matmuls), so the
        # full x lands ~2us earlier for second-half plane production.
        wrts = {}
        for c in range(NCH):
            sl = slice(c * BS, (c + 1) * BS)
            nc.sync.dma_start(xsb[:, sl], xt[c * 128:(c + 1) * 128, :])
            if c < 2:
                wt = wrpool.tile([128, O], BF16, tag="wr", name=f"wr{c}")
                nc.sync.dma_start(wt[:], wres[c * 128:(c + 1) * 128, :])
                wrts[c] = wt
            nc.gpsimd.dma_start(fres[:, c:c + 1, :], xt[c * 128:(c + 1) * 128, :])
            nc.vector.tensor_scalar(xc[:, sl], xsb[:, sl], -1.0, 1.0,
                                    AL.max, AL.min)
        for c in range(2, NCH):
            wt = wrpool.tile([128, O], BF16, tag="wr", name=f"wr{c}")
            nc.sync.dma_start(wt[:], wres[c * 128:(c + 1) * 128, :])
            wrts[c] = wt

        # ---- 11 spline planes -> fp8 via cast-DMA. Production (and PE
        # consumption) runs in PERM order: the cheap b-side-free k=10 plane
        # sits mid-stream as production catch-up slack. ----
        PERM = list(range(K))
        planes = {}
        for k in PERM:
            fk = fpool.tile([128, NCH, BS], FP8, tag="fk", name=f"fk{k}")
            has_b = k not in (0, 10)
            nsub = 2
            sw = FD // nsub
            if k <= 1:
                fn, sc, bi = (AF.Abs, 4.0, float(5 - k)) if k > 1 else \
                             (AF.Copy, 4.0, float(5 - k))
            elif k >= 9:
                fn, sc, bi = AF.Copy, -4.0, float(k - 5)
            else:
                fn, sc, bi = AF.Abs, 4.0, float(5 - k)
            for su in range(nsub):
                sl = slice(su * sw, (su + 1) * sw)
                csl = slice(su * (NCH // nsub), (su + 1) * (NCH // nsub))
                dT = dpool.tile([128, sw], BF16, tag=f"dT{nsub}", name="dT")
                nc.scalar.activation(dT[:], xc[:, sl], fn, bias=bi, scale=sc)
                a1n = tpool.tile([128, sw], BF16, tag=f"a1n{nsub}", name="a1n")
                nc.vector.tensor_scalar(a1n[:], dT[:], 2.0, 0.0, AL.subtract, AL.min)
                if k in ACT3:
                    a2u = dpool.tile([128, sw], BF16, tag=f"a2u{nsub}", name="a2u")
                    nc.scalar.activation(a2u[:], dT[:], AF.Square, bias=-2.0, scale=1.0)
                else:
                    a2u = tpool.tile([128, sw], BF16, tag=f"a2{nsub}", name="a2u")
                    nc.vector.tensor_tensor(a2u[:], a1n[:], a1n[:], AL.mult)
                An = tpool.tile([128, sw], BF16, tag=f"An{nsub}", name="An")
                nc.vector.tensor_tensor(An[:], a2u[:], a1n[:], AL.mult)
                if has_b:
                    b2u = dpool.tile([128, sw], BF16, tag=f"b2u{nsub}", name="b2u")
                    nc.scalar.activation(b2u[:], dT[:], AF.Square, bias=2.0, scale=-2.0)
                    bn = tpool.tile([128, sw], BF16, tag=f"bn{nsub}", name="bn")
                    nc.vector.tensor_scalar(bn[:], dT[:], 1.0, 0.0, AL.subtract, AL.min)
                    Bc = tpool.tile([128, sw], BF16, tag=f"Bc{nsub}", name="Bc")
                    nc.vector.tensor_tensor(Bc[:], b2u[:], bn[:], AL.mult)
                    G = gpool.tile([128, sw], BF16, tag=f"G{nsub}", name="G")
                    nc.vector.tensor_tensor(G[:], An[:], Bc[:], AL.subtract)
                else:
                    G = An
                nc.gpsimd.dma_start(fk[:, csl, :], G[:])  # bf16 -> fp8 cast
            planes[k] = fk

        # ---- matmul: [128 b x 512 o] x (4 bc x 2 oh) = 8 PSUM banks.
        # 4 residual chunks cover PE warmup; the other 4 act as filler
        # between early spline plane-groups (absorb plane-production lag). ----
        ps = [pspool.tile([128, 512], F32, name=f"ps{j}", tag=f"ps{j}")
              for j in range(8)]

        def resid_mms(c, start):
            wt = wrts[c]
            for bc in range(4):
                lhsT = fres[:, c:c + 1, bc * 128:(bc + 1) * 128]
                for oh in range(2):
                    nc.tensor.matmul(ps[bc * 2 + oh][:], lhsT,
                                     wt[:, oh * 512:(oh + 1) * 512],
                                     start=start, stop=False)

        for c in range(NCH):
            resid_mms(c, c == 0)
        filler = {}
        for pos, kk in enumerate(PERM):
            for cp in range(NCH // 2):
                j = kk * (NCH // 2) + cp
                last = (pos == len(PERM) - 1) and (cp == NCH // 2 - 1)
                wt = wspool.tile([128, 2, O], FP8, tag="ws")
                nc.sync.dma_start(
                    wt[:], wsp[j * 128:(j + 1) * 128, :]
                    .rearrange("p (two o) -> p two o", two=2))
                src = planes[kk]
                for bc in range(4):
                    lhsT = src[:, 2 * cp:2 * cp + 2, bc * 128:(bc + 1) * 128]
                    for oh in range(2):
                        nc.tensor.matmul(ps[bc * 2 + oh][:], lhsT,
                                         wt[:, :, oh * 512:(oh + 1) * 512],
                                         start=False, stop=last,
                                         perf_mode=DR)

        # ---- epilogue: PSUM * (1/SCALE) -> SBUF -> HBM ----
        for bc in range(4):
            for oh in range(2):
                obh = opool.tile([128, 512], F32, tag="ob", name=f"ob{bc}{oh}")
                if oh == 0:
                    nc.scalar.mul(obh[:], ps[bc * 2 + oh][:], 1.0 / SCALE)
                else:
                    nc.vector.tensor_scalar(obh[:], ps[bc * 2 + oh][:],
                                            1.0 / SCALE, None, AL.mult)
                nc.sync.dma_start(
                    out[bc * 128:(bc + 1) * 128, oh * 512:(oh + 1) * 512],
                    obh[:])

    nc.compile()
    _dedupe_ldweights(nc, mybir)
    return nc


def _dedupe_ldweights(nc, mybir):
    """Drop an Ldweights that reloads the exact same weights as the previous
    Ldweights on the PE stream with only Matmults in between (the oh=0/oh=1
    pair shares its stationary operand). Bail on any with sync_info."""
    import json as _json
    for fn in nc.m.functions:
        for blk in fn.blocks:
            insts = list(blk.instructions)
            kept = []
            last_key = None
            removed = 0
            for inst in insts:
                if inst.engine != mybir.EngineType.PE:
                    kept.append(inst)
                    continue
                op = type(inst).__name__
                if op == "InstLdweights":
                    si = inst.sync_info
                    has_sync = bool(si and (si.on_wait or si.on_update))
                    key = _json.dumps(
                        _json.loads(mybir.instruction_to_pretty_json_string(inst))
                        .get("ins"), sort_keys=True)
                    if key == last_key and not has_sync:
                        removed += 1
                        continue
                    last_key = key
                    kept.append(inst)
                elif op == "InstMatmult":
                    kept.append(inst)
                else:
                    last_key = None
                    kept.append(inst)
            if removed:
                blk.instructions = kept
    return nc


def _fold_weights(spline_coeffs, residual_weight, residual_bias, scale_base):
    scale = scale_base.astype(np.float32).mean(axis=1)                # [O]
    Ws = spline_coeffs.astype(np.float32) * scale[:, None, None]      # [O,I,K]
    Ws += residual_bias.astype(np.float32)[:, None, None] / I
    Ws *= -SCALE / 6.0            # device planes are -(6*basis_k)
    Ws = np.ascontiguousarray(Ws.transpose(2, 1, 0))                  # [K,I,O]
    # pair layout for DoubleRow moving operand: [44, 128, 2, O] rows
    Wsp = Ws.reshape(NPAIRS, 2, 128, O).transpose(0, 2, 1, 3).reshape(NPAIRS * 128, 2 * O)
    Wsp = np.clip(Wsp, -240.0, 240.0).astype(F8)
    Wr = np.ascontiguousarray(residual_weight.astype(np.float32).T * SCALE)
    Wr = Wr.astype(ml_dtypes.bfloat16)                                # [I,O]
    return Wsp, Wr


def _make_in_maps(inputs):
    Wsp, Wr = _fold_weights(inputs["spline_coeffs"], inputs["residual_weight"],
                            inputs["residual_bias"], inputs["scale_base"])
    x = np.asarray(inputs["x"], dtype=np.float32)
    in_maps = []
    for c in range(NCORES):
        xs = np.ascontiguousarray(x[c * BS:(c + 1) * BS, :].T)  # [I, BS]
        in_maps.append({"xt": xs, "wsp": Wsp, "wres": Wr})
    return in_maps


def kernel(x, spline_coeffs, residual_weight, residual_bias, scale_base):
    from concourse.bass_utils import run_bass_kernel_spmd

    if "nc" not in _cache:
        _cache["nc"] = _build_bass()
    nc = _cache["nc"]

    in_maps = _make_in_maps(dict(x=x, spline_coeffs=spline_coeffs,
                                 residual_weight=residual_weight,
                                 residual_bias=residual_bias,
                                 scale_base=scale_base))
    res = run_bass_kernel_spmd(nc, in_maps, core_ids=list(range(NCORES)))
    out = np.concatenate([r["out"] for r in res.results], axis=0)
    return out.astype(np.float32)

